# revision 10
# baseline (speedup 1.0000x reference)
"""PointerNetwork (teacher-forcing) Trainium2 Bass kernel.

Sharding: data-parallel over batch. B=32 examples across 8 cores -> 4/core.
Each core runs the full pipeline for its 4 examples:
  enc_linear -> encoder GRU scan -> decoder-input gather (via onehot matmul)
  -> decoder GRU scan -> Bahdanau additive attention with step mask.
No collectives; host concatenates the per-core [4,128,128] logits.

Layouts (per core, b = 4 local examples):
  xT      [2, 512]          x[b,t,i] at [i, 4t+b]
  encT    [h(2x128), 512]   enc_in.T, cols (t,b)
  encNat  [128t, (b, 256h)] enc_in natural, per example
  GI      [(t,b)=512, 768]  x @ WihT (+bih), 4 tiles of 128 rows
  EncOutT [h(2x128), (t,b)] hidden states transposed (also the scan state)
  KT/QT   [a(2x128), (b, t)]
Attention: T[a, (32k,128q)] = KT + QT broadcast-add, tanh on ACT,
  v-reduction via matmul with v placed in column r of the lhsT ("vdiag"),
  giving score rows [32, 512] per example; PE transposes + a mask matmul
  (lhsT = -1e9 * strict-lower-tri, rhs = onehot(targets)) accumulate the
  final [q, k] logits in PSUM.
"""

import sys

sys.path.insert(0, "/opt/trn_rl_repo")

from contextlib import ExitStack

import numpy as np

import concourse.bass as bass
import concourse.tile as tile
from concourse import bacc
from concourse import mybir
from concourse.bass_utils import run_bass_kernel_spmd

F32 = mybir.dt.float32
I32 = mybir.dt.int32
AF = mybir.ActivationFunctionType
OP = mybir.AluOpType

B, L, I, H, A = 32, 128, 2, 256, 256
NC = 8
PB = B // NC  # batch per core = 4
G3 = 3 * H  # 768
TB = L * PB  # 512 (t,b) rows per core

_CACHE = {}


def _build(bias_flags):
    (benc_nz, bgi_e_nz, bhhn_e_nz, bgi_d_nz, bhhn_d_nz) = bias_flags
    nc = bacc.Bacc("TRN2", target_bir_lowering=False, debug=False)

    def din(name, shape, dt=F32):
        return nc.dram_tensor(name, shape, dt, kind="ExternalInput").ap()

    xT = din("xT", [I, TB])
    tgT = din("tgT", [L, PB], I32)
    tgTr = din("tgTr", [L, PB], I32)
    wencT = din("wencT", [I, H])
    wihTe = din("wihTe", [H, G3])
    whhTe = din("whhTe", [H, G3])
    wihTd = din("wihTd", [H, G3])
    whhTd = din("whhTd", [H, G3])
    wkT = din("wkT", [H, A])
    wqT = din("wqT", [H, A])
    vdg = din("vd", [2, 128, 1024])
    iota = din("iota", [128, 128])
    iotaC = din("iotaC", [128, 1])
    idn = din("idn", [128, 128])
    ltneg = din("ltneg", [128, 128])
    benc = din("benc", [1, H])
    bgi_e = din("bgi_e", [1, G3])
    bhhn_e = din("bhhn_e", [1, H])
    bgi_d = din("bgi_d", [1, G3])
    bhhn_d = din("bhhn_d", [1, H])
    ones = din("ones", [1, TB])
    OUT = nc.dram_tensor("out", [PB, L, L], F32, kind="ExternalOutput").ap()

    with tile.TileContext(nc) as tc:
        with ExitStack() as ctx:
            cst = ctx.enter_context(tc.tile_pool(name="cst", bufs=1))

            _ld = [0]

            def load(ap, shape, dt=F32):
                _ld[0] += 1
                nm = f"c{ap.name}_{_ld[0]}"
                t = cst.tile(shape, dt, tag=nm, name=nm)
                nc.sync.dma_start(t[:], ap)
                return t

            xT_s = load(xT[:], [I, TB])
            wencT_s = load(wencT[:], [I, H])
            wihTe_s = [load(wihTe[128 * c : 128 * (c + 1), :], [128, G3]) for c in range(2)]
            whhTe_s = [load(whhTe[128 * c : 128 * (c + 1), :], [128, G3]) for c in range(2)]
            wihTd_s = [load(wihTd[128 * c : 128 * (c + 1), :], [128, G3]) for c in range(2)]
            whhTd_s = [load(whhTd[128 * c : 128 * (c + 1), :], [128, G3]) for c in range(2)]
            wkT_s = [load(wkT[128 * c : 128 * (c + 1), :], [128, A]) for c in range(2)]
            wqT_s = [load(wqT[128 * c : 128 * (c + 1), :], [128, A]) for c in range(2)]
            vd_s = [load(vdg[c], [128, 1024]) for c in range(2)]
            iota_s = load(iota[:], [128, 128])
            iotaC_s = load(iotaC[:], [128, 1])
            idn_s = load(idn[:], [128, 128])
            ltneg_s = load(ltneg[:], [128, 128])
            ones_s = load(ones[:], [1, TB])
            benc_s = load(benc[:], [1, H])
            bgi_e_s = load(bgi_e[:], [1, G3])
            bhhn_e_s = load(bhhn_e[:], [1, H])
            bgi_d_s = load(bgi_d[:], [1, G3])
            bhhn_d_s = load(bhhn_d[:], [1, H])

            tgT_i = load(tgT[:], [L, PB], I32)
            tgTr_i = load(tgTr[:], [L, PB], I32)
            tgT_f = cst.tile([L, PB], F32)
            nc.vector.tensor_copy(tgT_f[:], tgT_i[:])

            # persistent activation tensors
            encT = [cst.tile([128, TB], F32, tag=f"encT{c}", name=f"encT{c}") for c in range(2)]
            encNat = cst.tile([128, PB, H], F32)
            decT = cst.tile([128, 2, TB], F32)
            GIe = [cst.tile([128, G3], F32, tag=f"gie{m}", name=f"gie{m}") for m in range(4)]
            GId = [cst.tile([128, G3], F32, tag=f"gid{m}", name=f"gid{m}") for m in range(4)]
            EncOutT = cst.tile([128, 2, TB], F32)
            DecOutT = cst.tile([128, 2, TB], F32)
            onehotT_dec = cst.tile([128, PB, 128], F32)
            onehot_tg = cst.tile([128, PB, 128], F32)
            KT = cst.tile([128, PB, 2, 128], F32)
            QT = cst.tile([128, PB, 2, 128], F32)

            # ---------------- P1: encoder linear ----------------
            with tc.tile_pool(name="ps1", bufs=1, space="PSUM") as ps1p, \
                 tc.tile_pool(name="sb1", bufs=2) as sb1p:
                for hc in range(2):
                    ps = ps1p.tile([128, TB], F32, tag="pe")
                    nc.tensor.matmul(ps[:], wencT_s[:, 128 * hc : 128 * (hc + 1)],
                                     xT_s[:], start=True, stop=not benc_nz)
                    if benc_nz:
                        nc.tensor.matmul(ps[:], benc_s[:, 128 * hc : 128 * (hc + 1)],
                                         ones_s[:], start=False, stop=True)
                    nc.vector.tensor_copy(encT[hc][:], ps[:])
                for b in range(PB):
                    ps = ps1p.tile([128, H], F32, tag="pn")
                    nc.tensor.matmul(ps[:], xT_s[:, b : 4 * (L - 1) + b + 1 : 4],
                                     wencT_s[:], start=True, stop=not benc_nz)
                    if benc_nz:
                        nc.tensor.matmul(ps[:], ones_s[:, 0:128], benc_s[:],
                                         start=False, stop=True)
                    nc.vector.tensor_copy(encNat[:, b, :], ps[:])

                # ---------------- P2a: GI_e ----------------
                for m in range(4):
                    psa = ps1p.tile([128, 512], F32, tag="ga")
                    psb = ps1p.tile([128, 256], F32, tag="gb")
                    for hc in range(2):
                        lhs = encT[hc][:, 128 * m : 128 * (m + 1)]
                        nc.tensor.matmul(psa[:], lhs, wihTe_s[hc][:, 0:512],
                                         start=(hc == 0), stop=(hc == 1 and not bgi_e_nz))
                        nc.tensor.matmul(psb[:], lhs, wihTe_s[hc][:, 512:768],
                                         start=(hc == 0), stop=(hc == 1 and not bgi_e_nz))
                    if bgi_e_nz:
                        nc.tensor.matmul(psa[:], ones_s[:, 0:128], bgi_e_s[:, 0:512],
                                         start=False, stop=True)
                        nc.tensor.matmul(psb[:], ones_s[:, 0:128], bgi_e_s[:, 512:768],
                                         start=False, stop=True)
                    nc.vector.tensor_copy(GIe[m][:, 0:512], psa[:])
                    nc.vector.tensor_copy(GIe[m][:, 512:768], psb[:])

                # ---------------- P2b: decoder inputs ----------------
                for b in range(PB):
                    dbc_i = sb1p.tile([128, 128], I32, tag="dbci")
                    nc.sync.dma_start(dbc_i[:], tgTr[:, b].unsqueeze(0).broadcast_to([128, L]))
                    dbc_f = sb1p.tile([128, 128], F32, tag="dbcf")
                    nc.vector.tensor_copy(dbc_f[:], dbc_i[:])
                    nc.vector.tensor_scalar(out=onehotT_dec[:, b, :], in0=dbc_f[:],
                                            scalar1=iotaC_s[:], scalar2=None,
                                            op0=OP.is_equal)
                    nc.vector.tensor_scalar(out=onehot_tg[:, b, :], in0=iota_s[:],
                                            scalar1=tgT_f[:, b : b + 1], scalar2=None,
                                            op0=OP.is_equal)
                    for hc in range(2):
                        ps = ps1p.tile([128, 128], F32, tag="pd")
                        nc.tensor.matmul(ps[:], encNat[:, b, 128 * hc : 128 * (hc + 1)],
                                         onehotT_dec[:, b, :], start=True, stop=True)
                        nc.vector.tensor_copy(decT[:, hc, b : 4 * (L - 1) + b + 1 : 4], ps[:])

                # ---------------- P2c: GI_d ----------------
                for m in range(4):
                    psa = ps1p.tile([128, 512], F32, tag="ga")
                    psb = ps1p.tile([128, 256], F32, tag="gb")
                    for hc in range(2):
                        lhs = decT[:, hc, 128 * m : 128 * (m + 1)]
                        nc.tensor.matmul(psa[:], lhs, wihTd_s[hc][:, 0:512],
                                         start=(hc == 0), stop=(hc == 1 and not bgi_d_nz))
                        nc.tensor.matmul(psb[:], lhs, wihTd_s[hc][:, 512:768],
                                         start=(hc == 0), stop=(hc == 1 and not bgi_d_nz))
                    if bgi_d_nz:
                        nc.tensor.matmul(psa[:], ones_s[:, 0:128], bgi_d_s[:, 0:512],
                                         start=False, stop=True)
                        nc.tensor.matmul(psb[:], ones_s[:, 0:128], bgi_d_s[:, 512:768],
                                         start=False, stop=True)
                    nc.vector.tensor_copy(GId[m][:, 0:512], psa[:])
                    nc.vector.tensor_copy(GId[m][:, 512:768], psb[:])

            # ---------------- GRU scans ----------------
            def gru_scan(GI, whh_s, bhhn_s, bhhn_nz, OutT, hT_init, hnat_init, sp, pp):
                """hT_init: None (zeros) or [tile, col] AP pair for initial state."""
                hnat_prev = hnat_init
                for t in range(L):
                    stage = sp.tile([PB, G3], F32, tag="stage", name="stage")
                    src = GI[t // 32][(4 * t) % 128 : (4 * t) % 128 + 4, :]
                    nc.gpsimd.dma_start(stage[:], src)
                    first = hT_init is None and t == 0
                    if not first:
                        if t == 0:
                            lhs = [hT_init[:, c, TB - 4 : TB] for c in range(2)]
                        else:
                            lhs = [OutT[:, c, 4 * (t - 1) : 4 * t] for c in range(2)]
                        ps_rz = pp.tile([PB, 512], F32, tag="rz", name="psrz")
                        ps_n = pp.tile([PB, 256], F32, tag="n", name="psn")
                        nc.tensor.matmul(ps_rz[:], lhs[0], whh_s[0][:, 0:512],
                                         start=True, stop=False)
                        nc.tensor.matmul(ps_rz[:], lhs[1], whh_s[1][:, 0:512],
                                         start=False, stop=True)
                        nc.tensor.matmul(ps_n[:], lhs[0], whh_s[0][:, 512:768],
                                         start=True, stop=False)
                        nc.tensor.matmul(ps_n[:], lhs[1], whh_s[1][:, 512:768],
                                         start=False, stop=not bhhn_nz)
                        if bhhn_nz:
                            nc.tensor.matmul(ps_n[:], ones_s[:, 0:PB], bhhn_s[:],
                                             start=False, stop=True)
                        srz = sp.tile([PB, 512], F32, tag="srz", name="srz")
                        nc.vector.tensor_tensor(out=srz[:], in0=stage[:, 0:512],
                                                in1=ps_rz[:], op=OP.add)
                        rz = sp.tile([PB, 512], F32, tag="rzs", name="rzs")
                        nc.scalar.activation(rz[:], srz[:], AF.Sigmoid)
                        m1 = sp.tile([PB, 256], F32, tag="m1", name="m1")
                        nc.vector.tensor_tensor(out=m1[:], in0=rz[:, 0:256],
                                                in1=ps_n[:], op=OP.mult)
                        m2 = sp.tile([PB, 256], F32, tag="m2", name="m2")
                        nc.vector.tensor_tensor(out=m2[:], in0=m1[:],
                                                in1=stage[:, 512:768], op=OP.add)
                        n_sb = sp.tile([PB, 256], F32, tag="nsb", name="nsb")
                        nc.scalar.activation(n_sb[:], m2[:], AF.Tanh)
                        d = sp.tile([PB, 256], F32, tag="d", name="d")
                        nc.vector.tensor_tensor(out=d[:], in0=hnat_prev[:],
                                                in1=n_sb[:], op=OP.subtract)
                        e = sp.tile([PB, 256], F32, tag="e", name="e")
                        nc.vector.tensor_tensor(out=e[:], in0=rz[:, 256:512],
                                                in1=d[:], op=OP.mult)
                        hN = sp.tile([PB, 256], F32, tag="hn", name="hn")
                        nc.vector.tensor_tensor(out=hN[:], in0=n_sb[:], in1=e[:],
                                                op=OP.add)
                    else:
                        # h == 0: r,z = sigmoid(gi_rz); n = tanh(gi_n); h' = n - z*n
                        rz = sp.tile([PB, 512], F32, tag="rzs", name="rzs")
                        nc.scalar.activation(rz[:], stage[:, 0:512], AF.Sigmoid)
                        n_sb = sp.tile([PB, 256], F32, tag="nsb", name="nsb")
                        nc.scalar.activation(n_sb[:], stage[:, 512:768], AF.Tanh)
                        e = sp.tile([PB, 256], F32, tag="e", name="e")
                        nc.vector.tensor_tensor(out=e[:], in0=rz[:, 256:512],
                                                in1=n_sb[:], op=OP.mult)
                        hN = sp.tile([PB, 256], F32, tag="hn", name="hn")
                        nc.vector.tensor_tensor(out=hN[:], in0=n_sb[:], in1=e[:],
                                                op=OP.subtract)
                    ps_hT = pp.tile([128, 2 * PB], F32, tag="ht", name="psht")
                    nc.tensor.matmul(ps_hT[:, 0:PB], hN[:, 0:128], idn_s[0:PB, 0:PB],
                                     is_transpose=True, start=True, stop=False,
                                     skip_group_check=True)
                    nc.tensor.matmul(ps_hT[:, PB : 2 * PB], hN[:, 128:256],
                                     idn_s[0:PB, 0:PB], is_transpose=True,
                                     start=False, stop=True, skip_group_check=True)
                    nc.vector.tensor_copy(
                        OutT[:, :, 4 * t : 4 * (t + 1)],
                        ps_hT[:].rearrange("p (c b) -> p c b", c=2))
                    hnat_prev = hN
                return hnat_prev

            with tc.tile_pool(name="spe", bufs=4) as spe, \
                 tc.tile_pool(name="ppe", bufs=2, space="PSUM") as ppe:
                hnat_enc = gru_scan(GIe, whhTe_s, bhhn_e_s, bhhn_e_nz, EncOutT,
                                    None, None, spe, ppe)
                gru_scan(GId, whhTd_s, bhhn_d_s, bhhn_d_nz, DecOutT,
                         EncOutT, hnat_enc, spe, ppe)

            # ---------------- projections K, Q ----------------
            with tc.tile_pool(name="psp", bufs=2, space="PSUM") as psp:
                for (w_s, srcT, dstT) in ((wkT_s, EncOutT, KT), (wqT_s, DecOutT, QT)):
                    for b in range(PB):
                        for ac in range(2):
                            ps = psp.tile([128, 128], F32, tag="pk")
                            for hc in range(2):
                                nc.tensor.matmul(
                                    ps[:], w_s[hc][:, 128 * ac : 128 * (ac + 1)],
                                    srcT[:, hc, b : 4 * (L - 1) + b + 1 : 4],
                                    start=(hc == 0), stop=(hc == 1))
                            nc.vector.tensor_copy(dstT[:, b, ac, :], ps[:])

            # ---------------- attention ----------------
            with tc.tile_pool(name="atn", bufs=3) as atn, \
                 tc.tile_pool(name="atp", bufs=2, space="PSUM") as atp:
                for b in range(PB):
                    ps_sc = atp.tile([32, 512], F32, tag="sc")
                    for kb in range(4):
                        for c in range(2):
                            Tt = atn.tile([128, 32, 128], F32, tag="T")
                            in0 = KT[:, b, c, 32 * kb : 32 * (kb + 1)].unsqueeze(2) \
                                .broadcast_to([128, 32, 128])
                            in1 = QT[:, b, c, :].unsqueeze(1).broadcast_to([128, 32, 128])
                            nc.vector.tensor_tensor(out=Tt[:], in0=in0, in1=in1, op=OP.add)
                            nc.scalar.activation(Tt[:], Tt[:], AF.Tanh)
                            Tf = Tt[:].rearrange("p k q -> p (k q)")
                            for rl in range(8):
                                r = 8 * kb + rl
                                nc.tensor.matmul(
                                    ps_sc[:], vd_s[c][:, 32 * r : 32 * (r + 1)],
                                    Tf[:, 512 * rl : 512 * (rl + 1)],
                                    start=(kb == 0 and c == 0 and rl == 0),
                                    stop=(kb == 3 and c == 1 and rl == 7))
                    srow = atn.tile([32, 512], F32, tag="srow")
                    nc.vector.tensor_copy(srow[:], ps_sc[:])
                    # ps_out cols are k' = 32*(k%4) + k//4 (PE-T chunks contiguous);
                    # onehot_tg was built against the same permuted iota.
                    ps_out = atp.tile([128, 128], F32, tag="out")
                    for j in range(4):
                        nc.tensor.matmul(ps_out[:, 32 * j : 32 * (j + 1)],
                                         srow[:, 128 * j : 128 * (j + 1)],
                                         idn_s[0:32, 0:32], is_transpose=True,
                                         start=(j == 0), stop=False,
                                         skip_group_check=True)
                    nc.tensor.matmul(ps_out[:], ltneg_s[:], onehot_tg[:, b, :],
                                     start=False, stop=True, skip_group_check=True)
                    out_sb = atn.tile([128, 128], F32, tag="osb")
                    nc.vector.tensor_copy(
                        out_sb[:].rearrange("p (r j) -> p r j", r=32),
                        ps_out[:].rearrange("p (j r) -> p r j", j=4))
                    nc.sync.dma_start(OUT[b], out_sb[:])
    nc.compile()
    return nc


def _prep(inputs):
    x = np.asarray(inputs["inputs"], np.float32)
    tg = np.asarray(inputs["targets"])
    assert tg.dtype in (np.int32, np.int64)
    tg32 = tg.astype(np.int32)
    dec_idx = np.roll(tg32, 1, axis=1)

    W_enc = np.asarray(inputs["W_enc"], np.float32)
    b_enc = np.asarray(inputs["b_enc"], np.float32)
    v_att = np.asarray(inputs["v_att"], np.float32)

    def gi_fold(bih, bhh):
        bg = bih.copy()
        bg[: 2 * H] += bhh[: 2 * H]
        return bg.reshape(1, G3), bhh[2 * H :].reshape(1, H)

    bgi_e, bhhn_e = gi_fold(np.asarray(inputs["bih_e"], np.float32),
                            np.asarray(inputs["bhh_e"], np.float32))
    bgi_d, bhhn_d = gi_fold(np.asarray(inputs["bih_d"], np.float32),
                            np.asarray(inputs["bhh_d"], np.float32))

    vd = np.zeros((2, 128, 1024), np.float32)
    for c in range(2):
        for r in range(32):
            vd[c, :, 32 * r + r] = v_att[128 * c : 128 * (c + 1)]

    shared = {
        "wencT": np.ascontiguousarray(W_enc.T),
        "wihTe": np.ascontiguousarray(np.asarray(inputs["Wih_e"], np.float32).T),
        "whhTe": np.ascontiguousarray(np.asarray(inputs["Whh_e"], np.float32).T),
        "wihTd": np.ascontiguousarray(np.asarray(inputs["Wih_d"], np.float32).T),
        "whhTd": np.ascontiguousarray(np.asarray(inputs["Whh_d"], np.float32).T),
        "wkT": np.ascontiguousarray(np.asarray(inputs["Wk"], np.float32).T),
        "wqT": np.ascontiguousarray(np.asarray(inputs["Wq"], np.float32).T),
        "vd": vd,
        "iota": np.broadcast_to((4 * (np.arange(128) % 32) + np.arange(128) // 32).astype(np.float32), (128, 128)).copy(),
        "iotaC": np.arange(128, dtype=np.float32).reshape(128, 1),
        "idn": np.eye(128, dtype=np.float32),
        "ltneg": (-1e9 * (np.arange(128)[:, None] < np.arange(128)[None, :])).astype(np.float32),
        "benc": b_enc.reshape(1, H),
        "bgi_e": bgi_e, "bhhn_e": bhhn_e, "bgi_d": bgi_d, "bhhn_d": bhhn_d,
        "ones": np.ones((1, TB), np.float32),
    }
    bias_flags = tuple(bool(np.any(shared[k])) for k in
                       ("benc", "bgi_e", "bhhn_e", "bgi_d", "bhhn_d"))
    in_maps = []
    for c in range(NC):
        xc = x[PB * c : PB * (c + 1)]
        m = dict(shared)
        m["xT"] = np.ascontiguousarray(xc.transpose(2, 1, 0).reshape(I, TB))
        m["tgT"] = np.ascontiguousarray(tg32[PB * c : PB * (c + 1)].T)
        m["tgTr"] = np.ascontiguousarray(dec_idx[PB * c : PB * (c + 1)].T)
        in_maps.append(m)
    return in_maps, bias_flags, tg.dtype


def run(inputs, **kw):
    in_maps, bias_flags, _ = _prep(inputs)
    if bias_flags not in _CACHE:
        _CACHE[bias_flags] = _build(bias_flags)
    nc = _CACHE[bias_flags]
    res = run_bass_kernel_spmd(nc, in_maps, list(range(NC)), **kw)
    out = np.concatenate([res.results[c]["out"] for c in range(NC)], axis=0)
    return out.astype(np.float32), res


def kernel(**inputs):
    out, _ = run(inputs)
    return out


# revision 12
# speedup vs baseline: 1.0270x; 1.0270x over previous
"""PointerNetwork (teacher-forcing) Trainium2 Bass kernel.

Sharding: data-parallel over batch. B=32 examples across 8 cores -> 4/core.
Each core runs the full pipeline for its 4 examples:
  enc_linear -> encoder GRU scan -> decoder-input gather (via onehot matmul)
  -> decoder GRU scan -> Bahdanau additive attention with step mask.
No collectives; host concatenates the per-core [4,128,128] logits.

Layouts (per core, b = 4 local examples):
  xT      [2, 512]          x[b,t,i] at [i, 4t+b]
  encT    [h(2x128), 512]   enc_in.T, cols (t,b)
  encNat  [128t, (b, 256h)] enc_in natural, per example
  GI      [(t,b)=512, 768]  x @ WihT (+bih), 4 tiles of 128 rows
  EncOutT [h(2x128), (t,b)] hidden states transposed (also the scan state)
  KT/QT   [a(2x128), (b, t)]
Attention: T[a, (32k,128q)] = KT + QT broadcast-add, tanh on ACT,
  v-reduction via matmul with v placed in column r of the lhsT ("vdiag"),
  giving score rows [32, 512] per example; PE transposes + a mask matmul
  (lhsT = -1e9 * strict-lower-tri, rhs = onehot(targets)) accumulate the
  final [q, k] logits in PSUM.
"""

import sys

sys.path.insert(0, "/opt/trn_rl_repo")

from contextlib import ExitStack

import numpy as np

import concourse.bass as bass
import concourse.tile as tile
from concourse import bacc
from concourse import mybir
from concourse.bass_utils import run_bass_kernel_spmd

F32 = mybir.dt.float32
I32 = mybir.dt.int32
AF = mybir.ActivationFunctionType
OP = mybir.AluOpType

B, L, I, H, A = 32, 128, 2, 256, 256
NC = 8
PB = B // NC  # batch per core = 4
G3 = 3 * H  # 768
TB = L * PB  # 512 (t,b) rows per core

_CACHE = {}


def _build(bias_flags):
    (benc_nz, bgi_e_nz, bhhn_e_nz, bgi_d_nz, bhhn_d_nz) = bias_flags
    nc = bacc.Bacc("TRN2", target_bir_lowering=False, debug=False)

    def din(name, shape, dt=F32):
        return nc.dram_tensor(name, shape, dt, kind="ExternalInput").ap()

    xT = din("xT", [I, TB])
    tgT = din("tgT", [L, PB], I32)
    tgTr = din("tgTr", [L, PB], I32)
    wencT = din("wencT", [I, H])
    wihTe = din("wihTe", [H, G3])
    whhTe = din("whhTe", [H, G3])
    wihTd = din("wihTd", [H, G3])
    whhTd = din("whhTd", [H, G3])
    wkT = din("wkT", [H, A])
    wqT = din("wqT", [H, A])
    vdg = din("vd", [2, 128, 1024])
    iota = din("iota", [128, 128])
    iotaC = din("iotaC", [128, 1])
    idn = din("idn", [128, 128])
    ltneg = din("ltneg", [128, 128])
    benc = din("benc", [1, H])
    bgi_e = din("bgi_e", [1, G3])
    bhhn_e = din("bhhn_e", [1, H])
    bgi_d = din("bgi_d", [1, G3])
    bhhn_d = din("bhhn_d", [1, H])
    ones = din("ones", [1, TB])
    OUT = nc.dram_tensor("out", [PB, L, L], F32, kind="ExternalOutput").ap()

    with tile.TileContext(nc) as tc:
        with ExitStack() as ctx:
            cst = ctx.enter_context(tc.tile_pool(name="cst", bufs=1))

            _ld = [0]

            def load(ap, shape, dt=F32):
                _ld[0] += 1
                nm = f"c{ap.name}_{_ld[0]}"
                t = cst.tile(shape, dt, tag=nm, name=nm)
                nc.sync.dma_start(t[:], ap)
                return t

            xT_s = load(xT[:], [I, TB])
            wencT_s = load(wencT[:], [I, H])
            wihTe_s = [load(wihTe[128 * c : 128 * (c + 1), :], [128, G3]) for c in range(2)]
            whhTe_s = [load(whhTe[128 * c : 128 * (c + 1), :], [128, G3]) for c in range(2)]
            wihTd_s = [load(wihTd[128 * c : 128 * (c + 1), :], [128, G3]) for c in range(2)]
            whhTd_s = [load(whhTd[128 * c : 128 * (c + 1), :], [128, G3]) for c in range(2)]
            wkT_s = [load(wkT[128 * c : 128 * (c + 1), :], [128, A]) for c in range(2)]
            wqT_s = [load(wqT[128 * c : 128 * (c + 1), :], [128, A]) for c in range(2)]
            vd_s = [load(vdg[c], [128, 1024]) for c in range(2)]
            iota_s = load(iota[:], [128, 128])
            iotaC_s = load(iotaC[:], [128, 1])
            idn_s = load(idn[:], [128, 128])
            ltneg_s = load(ltneg[:], [128, 128])
            ones_s = load(ones[:], [1, TB])
            benc_s = load(benc[:], [1, H])
            bgi_e_s = load(bgi_e[:], [1, G3])
            bhhn_e_s = load(bhhn_e[:], [1, H])
            bgi_d_s = load(bgi_d[:], [1, G3])
            bhhn_d_s = load(bhhn_d[:], [1, H])

            tgT_i = load(tgT[:], [L, PB], I32)
            tgTr_i = load(tgTr[:], [L, PB], I32)
            tgT_f = cst.tile([L, PB], F32)
            nc.vector.tensor_copy(tgT_f[:], tgT_i[:])

            # persistent activation tensors
            encT = [cst.tile([128, TB], F32, tag=f"encT{c}", name=f"encT{c}") for c in range(2)]
            encNat = cst.tile([128, PB, H], F32)
            decT = cst.tile([128, 2, TB], F32)
            GIe = [cst.tile([128, G3], F32, tag=f"gie{m}", name=f"gie{m}") for m in range(4)]
            GId = [cst.tile([128, G3], F32, tag=f"gid{m}", name=f"gid{m}") for m in range(4)]
            EncOutT = cst.tile([128, 2, TB], F32)
            DecOutT = cst.tile([128, 2, TB], F32)
            onehotT_dec = cst.tile([128, PB, 128], F32)
            onehot_tg = cst.tile([128, PB, 128], F32)
            KT = cst.tile([128, PB, 2, 128], F32)
            QT = cst.tile([128, PB, 2, 128], F32)

            # ---------------- P1: encoder linear ----------------
            with tc.tile_pool(name="ps1", bufs=1, space="PSUM") as ps1p, \
                 tc.tile_pool(name="sb1", bufs=2) as sb1p:
                for hc in range(2):
                    ps = ps1p.tile([128, TB], F32, tag="pe")
                    nc.tensor.matmul(ps[:], wencT_s[:, 128 * hc : 128 * (hc + 1)],
                                     xT_s[:], start=True, stop=not benc_nz)
                    if benc_nz:
                        nc.tensor.matmul(ps[:], benc_s[:, 128 * hc : 128 * (hc + 1)],
                                         ones_s[:], start=False, stop=True)
                    nc.vector.tensor_copy(encT[hc][:], ps[:])
                for b in range(PB):
                    ps = ps1p.tile([128, H], F32, tag="pn")
                    nc.tensor.matmul(ps[:], xT_s[:, b : 4 * (L - 1) + b + 1 : 4],
                                     wencT_s[:], start=True, stop=not benc_nz)
                    if benc_nz:
                        nc.tensor.matmul(ps[:], ones_s[:, 0:128], benc_s[:],
                                         start=False, stop=True)
                    nc.vector.tensor_copy(encNat[:, b, :], ps[:])

                # ---------------- P2a: GI_e ----------------
                for m in range(4):
                    psa = ps1p.tile([128, 512], F32, tag="ga")
                    psb = ps1p.tile([128, 256], F32, tag="gb")
                    for hc in range(2):
                        lhs = encT[hc][:, 128 * m : 128 * (m + 1)]
                        nc.tensor.matmul(psa[:], lhs, wihTe_s[hc][:, 0:512],
                                         start=(hc == 0), stop=(hc == 1 and not bgi_e_nz))
                        nc.tensor.matmul(psb[:], lhs, wihTe_s[hc][:, 512:768],
                                         start=(hc == 0), stop=(hc == 1 and not bgi_e_nz))
                    if bgi_e_nz:
                        nc.tensor.matmul(psa[:], ones_s[:, 0:128], bgi_e_s[:, 0:512],
                                         start=False, stop=True)
                        nc.tensor.matmul(psb[:], ones_s[:, 0:128], bgi_e_s[:, 512:768],
                                         start=False, stop=True)
                    nc.vector.tensor_copy(GIe[m][:, 0:512], psa[:])
                    nc.vector.tensor_copy(GIe[m][:, 512:768], psb[:])

                # ---------------- P2b: decoder inputs ----------------
                for b in range(PB):
                    dbc_i = sb1p.tile([128, 128], I32, tag="dbci")
                    nc.sync.dma_start(dbc_i[:], tgTr[:, b].unsqueeze(0).broadcast_to([128, L]))
                    dbc_f = sb1p.tile([128, 128], F32, tag="dbcf")
                    nc.vector.tensor_copy(dbc_f[:], dbc_i[:])
                    nc.vector.tensor_scalar(out=onehotT_dec[:, b, :], in0=dbc_f[:],
                                            scalar1=iotaC_s[:], scalar2=None,
                                            op0=OP.is_equal)
                    nc.vector.tensor_scalar(out=onehot_tg[:, b, :], in0=iota_s[:],
                                            scalar1=tgT_f[:, b : b + 1], scalar2=None,
                                            op0=OP.is_equal)
                    for hc in range(2):
                        ps = ps1p.tile([128, 128], F32, tag="pd")
                        nc.tensor.matmul(ps[:], encNat[:, b, 128 * hc : 128 * (hc + 1)],
                                         onehotT_dec[:, b, :], start=True, stop=True)
                        nc.vector.tensor_copy(decT[:, hc, b : 4 * (L - 1) + b + 1 : 4], ps[:])

                # ---------------- P2c: GI_d ----------------
                for m in range(4):
                    psa = ps1p.tile([128, 512], F32, tag="ga")
                    psb = ps1p.tile([128, 256], F32, tag="gb")
                    for hc in range(2):
                        lhs = decT[:, hc, 128 * m : 128 * (m + 1)]
                        nc.tensor.matmul(psa[:], lhs, wihTd_s[hc][:, 0:512],
                                         start=(hc == 0), stop=(hc == 1 and not bgi_d_nz))
                        nc.tensor.matmul(psb[:], lhs, wihTd_s[hc][:, 512:768],
                                         start=(hc == 0), stop=(hc == 1 and not bgi_d_nz))
                    if bgi_d_nz:
                        nc.tensor.matmul(psa[:], ones_s[:, 0:128], bgi_d_s[:, 0:512],
                                         start=False, stop=True)
                        nc.tensor.matmul(psb[:], ones_s[:, 0:128], bgi_d_s[:, 512:768],
                                         start=False, stop=True)
                    nc.vector.tensor_copy(GId[m][:, 0:512], psa[:])
                    nc.vector.tensor_copy(GId[m][:, 512:768], psb[:])

            # ---------------- GRU scans ----------------
            def gru_scan(GI, whh_s, bhhn_s, bhhn_nz, OutT, hT_init, hnat_init, sp, pp):
                """hT_init: None (zeros) or tile whose last column is the initial state."""
                hnat_prev = hnat_init
                for t in range(L):
                    stage = sp.tile([PB, G3], F32, tag="stage", name="stage")
                    srcgi = GI[t // 32][(4 * t) % 128 : (4 * t) % 128 + 4, :]
                    nc.gpsimd.dma_start(stage[:], srcgi)
                    first = hT_init is None and t == 0
                    ps_r = pp.tile([PB, 256], F32, tag="psr", name="psr")
                    ps_z = pp.tile([PB, 256], F32, tag="psz", name="psz")
                    # gi lands in PSUM via K=PB identity matmul (off critical path)
                    nc.tensor.matmul(ps_r[:], idn_s[0:PB, 0:PB], stage[:, 0:256],
                                     start=True, stop=first)
                    nc.tensor.matmul(ps_z[:], idn_s[0:PB, 0:PB], stage[:, 256:512],
                                     start=True, stop=first)
                    if not first:
                        if t == 0:
                            lhs = [hT_init[:, c, TB - 4 : TB] for c in range(2)]
                        else:
                            lhs = [OutT[:, c, 4 * (t - 1) : 4 * t] for c in range(2)]
                        ps_n = pp.tile([PB, 256], F32, tag="psn", name="psn")
                        nc.tensor.matmul(ps_r[:], lhs[0], whh_s[0][:, 0:256],
                                         start=False, stop=False)
                        nc.tensor.matmul(ps_r[:], lhs[1], whh_s[1][:, 0:256],
                                         start=False, stop=True)
                        nc.tensor.matmul(ps_z[:], lhs[0], whh_s[0][:, 256:512],
                                         start=False, stop=False)
                        nc.tensor.matmul(ps_z[:], lhs[1], whh_s[1][:, 256:512],
                                         start=False, stop=True)
                        nc.tensor.matmul(ps_n[:], lhs[0], whh_s[0][:, 512:768],
                                         start=True, stop=False)
                        nc.tensor.matmul(ps_n[:], lhs[1], whh_s[1][:, 512:768],
                                         start=False, stop=not bhhn_nz)
                        if bhhn_nz:
                            nc.tensor.matmul(ps_n[:], ones_s[:, 0:PB], bhhn_s[:],
                                             start=False, stop=True)
                        r_sb = sp.tile([PB, 256], F32, tag="rsb", name="rsb")
                        nc.scalar.activation(r_sb[:], ps_r[:], AF.Sigmoid)
                        z_sb = sp.tile([PB, 256], F32, tag="zsb", name="zsb")
                        nc.scalar.activation(z_sb[:], ps_z[:], AF.Sigmoid)
                        m1 = sp.tile([PB, 256], F32, tag="m1", name="m1")
                        nc.vector.tensor_tensor(out=m1[:], in0=r_sb[:],
                                                in1=ps_n[:], op=OP.mult)
                        m2 = sp.tile([PB, 256], F32, tag="m2", name="m2")
                        nc.vector.tensor_tensor(out=m2[:], in0=m1[:],
                                                in1=stage[:, 512:768], op=OP.add)
                        n_sb = sp.tile([PB, 256], F32, tag="nsb", name="nsb")
                        nc.scalar.activation(n_sb[:], m2[:], AF.Tanh)
                        d = sp.tile([PB, 256], F32, tag="d", name="d")
                        nc.vector.tensor_tensor(out=d[:], in0=hnat_prev[:],
                                                in1=n_sb[:], op=OP.subtract)
                        e = sp.tile([PB, 256], F32, tag="e", name="e")
                        nc.vector.tensor_tensor(out=e[:], in0=z_sb[:],
                                                in1=d[:], op=OP.mult)
                        hN = sp.tile([PB, 256], F32, tag="hn", name="hn")
                        nc.vector.tensor_tensor(out=hN[:], in0=n_sb[:], in1=e[:],
                                                op=OP.add)
                    else:
                        # h == 0: r,z = sigmoid(gi_rz); n = tanh(gi_n); h' = n - z*n
                        z_sb = sp.tile([PB, 256], F32, tag="zsb", name="zsb")
                        nc.scalar.activation(z_sb[:], ps_z[:], AF.Sigmoid)
                        n_sb = sp.tile([PB, 256], F32, tag="nsb", name="nsb")
                        nc.scalar.activation(n_sb[:], stage[:, 512:768], AF.Tanh)
                        e = sp.tile([PB, 256], F32, tag="e", name="e")
                        nc.vector.tensor_tensor(out=e[:], in0=z_sb[:], in1=n_sb[:],
                                                op=OP.mult)
                        hN = sp.tile([PB, 256], F32, tag="hn", name="hn")
                        nc.vector.tensor_tensor(out=hN[:], in0=n_sb[:], in1=e[:],
                                                op=OP.subtract)
                    ps_hT = pp.tile([128, 2 * PB], F32, tag="misc", name="psht")
                    nc.tensor.matmul(ps_hT[:, 0:PB], hN[:, 0:128], idn_s[0:PB, 0:PB],
                                     is_transpose=True, start=True, stop=False,
                                     skip_group_check=True)
                    nc.tensor.matmul(ps_hT[:, PB : 2 * PB], hN[:, 128:256],
                                     idn_s[0:PB, 0:PB], is_transpose=True,
                                     start=False, stop=True, skip_group_check=True)
                    nc.vector.tensor_copy(
                        OutT[:, :, 4 * t : 4 * (t + 1)],
                        ps_hT[:, 0 : 2 * PB].rearrange("p (c b) -> p c b", c=2))
                    hnat_prev = hN
                return hnat_prev

            with tc.tile_pool(name="spe", bufs=4) as spe, \
                 tc.tile_pool(name="ppe", bufs=2, space="PSUM") as ppe:
                hnat_enc = gru_scan(GIe, whhTe_s, bhhn_e_s, bhhn_e_nz, EncOutT,
                                    None, None, spe, ppe)
                gru_scan(GId, whhTd_s, bhhn_d_s, bhhn_d_nz, DecOutT,
                         EncOutT, hnat_enc, spe, ppe)

            # ---------------- projections K, Q ----------------
            with tc.tile_pool(name="psp", bufs=2, space="PSUM") as psp:
                for (w_s, srcT, dstT) in ((wkT_s, EncOutT, KT), (wqT_s, DecOutT, QT)):
                    for b in range(PB):
                        for ac in range(2):
                            ps = psp.tile([128, 128], F32, tag="pk")
                            for hc in range(2):
                                nc.tensor.matmul(
                                    ps[:], w_s[hc][:, 128 * ac : 128 * (ac + 1)],
                                    srcT[:, hc, b : 4 * (L - 1) + b + 1 : 4],
                                    start=(hc == 0), stop=(hc == 1))
                            nc.vector.tensor_copy(dstT[:, b, ac, :], ps[:])

            # ---------------- attention ----------------
            with tc.tile_pool(name="atn", bufs=3) as atn, \
                 tc.tile_pool(name="atp", bufs=2, space="PSUM") as atp:
                for b in range(PB):
                    ps_sc = atp.tile([32, 512], F32, tag="sc")
                    for kb in range(4):
                        for c in range(2):
                            Tt = atn.tile([128, 32, 128], F32, tag="T")
                            in0 = KT[:, b, c, 32 * kb : 32 * (kb + 1)].unsqueeze(2) \
                                .broadcast_to([128, 32, 128])
                            in1 = QT[:, b, c, :].unsqueeze(1).broadcast_to([128, 32, 128])
                            nc.vector.tensor_tensor(out=Tt[:], in0=in0, in1=in1, op=OP.add)
                            nc.scalar.activation(Tt[:], Tt[:], AF.Tanh)
                            Tf = Tt[:].rearrange("p k q -> p (k q)")
                            for rl in range(8):
                                r = 8 * kb + rl
                                nc.tensor.matmul(
                                    ps_sc[:], vd_s[c][:, 32 * r : 32 * (r + 1)],
                                    Tf[:, 512 * rl : 512 * (rl + 1)],
                                    start=(kb == 0 and c == 0 and rl == 0),
                                    stop=(kb == 3 and c == 1 and rl == 7))
                    srow = atn.tile([32, 512], F32, tag="srow")
                    nc.vector.tensor_copy(srow[:], ps_sc[:])
                    # ps_out cols are k' = 32*(k%4) + k//4 (PE-T chunks contiguous);
                    # onehot_tg was built against the same permuted iota.
                    ps_out = atp.tile([128, 128], F32, tag="out")
                    for j in range(4):
                        nc.tensor.matmul(ps_out[:, 32 * j : 32 * (j + 1)],
                                         srow[:, 128 * j : 128 * (j + 1)],
                                         idn_s[0:32, 0:32], is_transpose=True,
                                         start=(j == 0), stop=False,
                                         skip_group_check=True)
                    nc.tensor.matmul(ps_out[:], ltneg_s[:], onehot_tg[:, b, :],
                                     start=False, stop=True, skip_group_check=True)
                    out_sb = atn.tile([128, 128], F32, tag="osb")
                    nc.vector.tensor_copy(
                        out_sb[:].rearrange("p (r j) -> p r j", r=32),
                        ps_out[:].rearrange("p (j r) -> p r j", j=4))
                    nc.sync.dma_start(OUT[b], out_sb[:])
    nc.compile()
    return nc


def _prep(inputs):
    x = np.asarray(inputs["inputs"], np.float32)
    tg = np.asarray(inputs["targets"])
    assert tg.dtype in (np.int32, np.int64)
    tg32 = tg.astype(np.int32)
    dec_idx = np.roll(tg32, 1, axis=1)

    W_enc = np.asarray(inputs["W_enc"], np.float32)
    b_enc = np.asarray(inputs["b_enc"], np.float32)
    v_att = np.asarray(inputs["v_att"], np.float32)

    def gi_fold(bih, bhh):
        bg = bih.copy()
        bg[: 2 * H] += bhh[: 2 * H]
        return bg.reshape(1, G3), bhh[2 * H :].reshape(1, H)

    bgi_e, bhhn_e = gi_fold(np.asarray(inputs["bih_e"], np.float32),
                            np.asarray(inputs["bhh_e"], np.float32))
    bgi_d, bhhn_d = gi_fold(np.asarray(inputs["bih_d"], np.float32),
                            np.asarray(inputs["bhh_d"], np.float32))

    vd = np.zeros((2, 128, 1024), np.float32)
    for c in range(2):
        for r in range(32):
            vd[c, :, 32 * r + r] = v_att[128 * c : 128 * (c + 1)]

    shared = {
        "wencT": np.ascontiguousarray(W_enc.T),
        "wihTe": np.ascontiguousarray(np.asarray(inputs["Wih_e"], np.float32).T),
        "whhTe": np.ascontiguousarray(np.asarray(inputs["Whh_e"], np.float32).T),
        "wihTd": np.ascontiguousarray(np.asarray(inputs["Wih_d"], np.float32).T),
        "whhTd": np.ascontiguousarray(np.asarray(inputs["Whh_d"], np.float32).T),
        "wkT": np.ascontiguousarray(np.asarray(inputs["Wk"], np.float32).T),
        "wqT": np.ascontiguousarray(np.asarray(inputs["Wq"], np.float32).T),
        "vd": vd,
        "iota": np.broadcast_to((4 * (np.arange(128) % 32) + np.arange(128) // 32).astype(np.float32), (128, 128)).copy(),
        "iotaC": np.arange(128, dtype=np.float32).reshape(128, 1),
        "idn": np.eye(128, dtype=np.float32),
        "ltneg": (-1e9 * (np.arange(128)[:, None] < np.arange(128)[None, :])).astype(np.float32),
        "benc": b_enc.reshape(1, H),
        "bgi_e": bgi_e, "bhhn_e": bhhn_e, "bgi_d": bgi_d, "bhhn_d": bhhn_d,
        "ones": np.ones((1, TB), np.float32),
    }
    bias_flags = tuple(bool(np.any(shared[k])) for k in
                       ("benc", "bgi_e", "bhhn_e", "bgi_d", "bhhn_d"))
    in_maps = []
    for c in range(NC):
        xc = x[PB * c : PB * (c + 1)]
        m = dict(shared)
        m["xT"] = np.ascontiguousarray(xc.transpose(2, 1, 0).reshape(I, TB))
        m["tgT"] = np.ascontiguousarray(tg32[PB * c : PB * (c + 1)].T)
        m["tgTr"] = np.ascontiguousarray(dec_idx[PB * c : PB * (c + 1)].T)
        in_maps.append(m)
    return in_maps, bias_flags, tg.dtype


def run(inputs, **kw):
    in_maps, bias_flags, _ = _prep(inputs)
    if bias_flags not in _CACHE:
        _CACHE[bias_flags] = _build(bias_flags)
    nc = _CACHE[bias_flags]
    res = run_bass_kernel_spmd(nc, in_maps, list(range(NC)), **kw)
    out = np.concatenate([res.results[c]["out"] for c in range(NC)], axis=0)
    return out.astype(np.float32), res


def kernel(**inputs):
    out, _ = run(inputs)
    return out
